# revision 1
# baseline (speedup 1.0000x reference)
"""Trainium2 Bass kernel for nn_CWGDN (dense_cnn): LN -> temporal pin conv ->
dynamic depthwise conv (w/ pooled kernel-generator branch) -> gate -> temporal
pout conv + residual.

Sharding: 16 (b,t) instances over 8 cores (2 each). Two SPMD launches:
  L1: per-core slices [t0-1, t0+3) of x -> gated(t0), gated(t0+1)   (bf16)
  L2: host reshards gated with t-halo -> pout conv + residual -> out (fp32)

LayerNorm is folded into the pin matmul: x is pre-scaled by rsqrt(var+eps)
(per-pixel, via a DMA-broadcast row) and the -mu*r / ln_b rank-1 terms ride
as extra contraction rows of the moving operand.
"""
import sys

sys.path.insert(0, "/opt/trn_rl_repo")

import numpy as np
import ml_dtypes

import concourse.bass as bass
import concourse.tile as tile
from concourse import bacc, mybir
from concourse.bass_utils import run_bass_kernel_spmd

BF = ml_dtypes.bfloat16
F32 = mybir.dt.float32
BF16 = mybir.dt.bfloat16
AL = mybir.AluOpType
ACTF = mybir.ActivationFunctionType

B, T, C, H, W = 2, 8, 64, 128, 128
HID = 128
S = H * W  # 16384
K = 3
EPS = 1e-5

_cache = {}
TRACE = False
PROF = {}


def _pool_dims(l):
    # layers 0..2 at 64x64, 3..5 at 32x32
    return (64, 64) if l < 3 else (32, 32)


def _dw_taps(nc, out_pad_v, in_pad_v, hh, ww, kap, bap, first_scale=None):
    """out = dwconv3x3(in) + bias on zero-padded tiles.

    in_pad_v/out_pad_v: APs viewed (128, hh+2, ww+2). kap: (128, 9) scalar
    cols. bap: (128,1) bias or None. Writes interior of out only.
    """
    oi = out_pad_v[:, 1 : hh + 1, 1 : ww + 1]
    # center tap + bias in one tensor_scalar
    if bap is not None:
        nc.vector.tensor_scalar(oi, in_pad_v[:, 1 : hh + 1, 1 : ww + 1],
                                kap[:, 4:5], bap, op0=AL.mult, op1=AL.add)
    else:
        nc.vector.tensor_scalar(oi, in_pad_v[:, 1 : hh + 1, 1 : ww + 1],
                                kap[:, 4:5], None, op0=AL.mult)
    for ky in range(3):
        for kx in range(3):
            if ky == 1 and kx == 1:
                continue
            src = in_pad_v[:, ky : ky + hh, kx : kx + ww]
            nc.vector.scalar_tensor_tensor(
                oi, src, kap[:, ky * 3 + kx : ky * 3 + kx + 1], oi,
                op0=AL.mult, op1=AL.add)


def _build_l1():
    nc = bacc.Bacc("TRN2", target_bir_lowering=False, debug=False, num_devices=8)
    xh = nc.dram_tensor("xh", [4, C, S], BF16, kind="ExternalInput")
    v4b = nc.dram_tensor("v4b", [128, 4], F32, kind="ExternalInput")
    w1p = nc.dram_tensor("w1p", [2, 2, 128, 128], BF16, kind="ExternalInput")
    w1lo = nc.dram_tensor("w1lo", [2, 2, 70, 128], BF16, kind="ExternalInput")
    bw = nc.dram_tensor("bw", [128, 54], F32, kind="ExternalInput")
    bb = nc.dram_tensor("bb", [128, 6], F32, kind="ExternalInput")
    tokw = nc.dram_tensor("tokw", [9, 128, 128], F32, kind="ExternalInput")
    tokb = nc.dram_tensor("tokb", [128, 9], F32, kind="ExternalInput")
    dwb = nc.dram_tensor("dwb", [128, 1], F32, kind="ExternalInput")
    g_out = [nc.dram_tensor(f"g{j}", [128, S], BF16, kind="ExternalOutput")
             for j in range(2)]
    # internal DRAM scratch
    scr_s = nc.dram_tensor("scr_s", [2, 2, S], F32)
    scr_q = nc.dram_tensor("scr_q", [2, 2, S], F32)
    r_scr = nc.dram_tensor("r_scr", [4, S], BF16)
    mur_scr = nc.dram_tensor("mur_scr", [4, S], BF16)
    v_scr = nc.dram_tensor("v_scr", [4, S], BF16)
    x2d = nc.dram_tensor("x2d", [2, 128, S], BF16)

    with tile.TileContext(nc, pool_alloc_mode="queue") as tc:
        with tc.tile_pool(name="wp", bufs=1) as wp:
            w1p_sb, w1lo_sb = [], []
            for j in range(2):
                w1p_sb.append([])
                w1lo_sb.append([])
                for oh in range(2):
                    tp = wp.tile([128, 128], BF16, tag=f"w1p{j}{oh}")
                    nc.sync.dma_start(tp[:], w1p[j, oh])
                    w1p_sb[j].append(tp)
                    tl = wp.tile([70, 128], BF16, tag=f"w1lo{j}{oh}")
                    nc.sync.dma_start(tl[:], w1lo[j, oh])
                    w1lo_sb[j].append(tl)
            bw_sb = wp.tile([128, 54], F32, tag="bw")
            nc.sync.dma_start(bw_sb[:], bw[:])
            bb_sb = wp.tile([128, 6], F32, tag="bb")
            nc.sync.dma_start(bb_sb[:], bb[:])
            tokw_sb = []
            for k in range(9):
                tk = wp.tile([128, 128], F32, tag=f"tokw{k}")
                nc.sync.dma_start(tk[:], tokw[k])
                tokw_sb.append(tk)
            tokb_sb = wp.tile([128, 9], F32, tag="tokb")
            nc.sync.dma_start(tokb_sb[:], tokb[:])
            dwb_sb = wp.tile([128, 1], F32, tag="dwb")
            nc.sync.dma_start(dwb_sb[:], dwb[:])
            v4_sb = wp.tile([128, 4], F32, tag="v4")
            nc.sync.dma_start(v4_sb[:], v4b[:])
            i2 = wp.tile([128, 2], BF16, tag="i2")
            nc.gpsimd.memset(i2[:, :], 0.0)
            nc.gpsimd.memset(i2[0:64, 0:1], 1.0)
            nc.gpsimd.memset(i2[64:128, 1:2], 1.0)
            eps_t = wp.tile([128, 1], F32, tag="eps")
            nc.gpsimd.memset(eps_t[:, :], EPS)

            with tc.tile_pool(name="cp0", bufs=1) as cp0:
                x1p = [cp0.tile([128, 130 * 130], BF16, tag=f"x1_{j}",
                                name=f"x1t{j}") for j in range(2)]
                pairs = []
                with tc.tile_pool(name="pp", bufs=1) as pp:
                    with tc.tile_pool(name="ap", bufs=1) as ap, \
                         tc.tile_pool(name="ps_a", bufs=1, space="PSUM") as psa:
                        for p in range(2):
                            xs = pp.tile([128, S], BF16, tag=f"pair{p}")
                            pairs.append(xs)
                            nc.sync.dma_start(
                                xs[:],
                                xh[2 * p : 2 * p + 2].rearrange("s c f -> (s c) f"))
                            # stats: per-pixel channel sums of x and x^2
                            for q in range(8):
                                n0 = q * 2048
                                sq = ap.tile([128, 2048], BF16, tag="sq", bufs=2)
                                nc.scalar.activation(sq[:], xs[:, n0 : n0 + 2048],
                                                     ACTF.Square)
                                psS = psa.tile([2, 2048], F32, tag="stS")
                                psQ = psa.tile([2, 2048], F32, tag="stQ")
                                for c in range(4):
                                    cs = slice(c * 512, (c + 1) * 512)
                                    nc.tensor.matmul(psS[:, cs], i2[:],
                                                     xs[:, n0 + c * 512 : n0 + (c + 1) * 512],
                                                     start=True, stop=True)
                                    nc.tensor.matmul(psQ[:, cs], i2[:],
                                                     sq[:, cs],
                                                     start=True, stop=True)
                                stgS = ap.tile([2, 2048], F32, tag="stgS", bufs=1)
                                nc.scalar.copy(stgS[:], psS[:])
                                nc.sync.dma_start(scr_s[p, :, n0 : n0 + 2048], stgS[:])
                                stgQ = ap.tile([2, 2048], F32, tag="stgQ", bufs=1)
                                nc.vector.tensor_copy(stgQ[:], psQ[:])
                                nc.sync.dma_start(scr_q[p, :, n0 : n0 + 2048], stgQ[:])
                            # r / mur in pixel-spread layout (128, 256)
                            sp_s = ap.tile([128, 256], F32, tag="sp_s")
                            nc.sync.dma_start(
                                sp_s[:], scr_s[p].rearrange("g (q f) -> q g f", q=128))
                            sp_q = ap.tile([128, 256], F32, tag="sp_q")
                            nc.sync.dma_start(
                                sp_q[:], scr_q[p].rearrange("g (q f) -> q g f", q=128))
                            mu = ap.tile([128, 256], F32, tag="mu")
                            nc.scalar.mul(mu[:], sp_s[:], 1.0 / 64.0)
                            msq = ap.tile([128, 256], F32, tag="msq")
                            nc.vector.tensor_tensor(msq[:], mu[:], mu[:], op=AL.mult)
                            nc.vector.scalar_tensor_tensor(
                                sp_q[:], sp_q[:], 1.0 / 64.0, msq[:],
                                op0=AL.mult, op1=AL.subtract)  # var -> sp_q
                            nc.scalar.activation(sp_s[:], sp_q[:], ACTF.Sqrt,
                                                 bias=eps_t[:, :])  # std -> sp_s
                            nc.vector.reciprocal(sp_q[:], sp_s[:])  # r -> sp_q
                            for g in range(2):
                                nc.vector.tensor_scalar(
                                    sp_q[:, g * 128 : (g + 1) * 128],
                                    sp_q[:, g * 128 : (g + 1) * 128],
                                    v4_sb[:, 2 * p + g : 2 * p + g + 1], None,
                                    op0=AL.mult)
                            r16 = ap.tile([128, 256], BF16, tag="r16")
                            nc.vector.tensor_copy(r16[:], sp_q[:])
                            nc.vector.tensor_tensor(mu[:], mu[:], sp_q[:],
                                                    op=AL.mult)  # mur -> mu
                            mur16 = ap.tile([128, 256], BF16, tag="mur16")
                            nc.vector.tensor_copy(mur16[:], mu[:])
                            vr16 = ap.tile([128, 256], BF16, tag="vr16")
                            nc.gpsimd.memset(vr16[:, :], 1.0)
                            for g in range(2):
                                nc.vector.tensor_scalar(
                                    vr16[:, g * 128 : (g + 1) * 128],
                                    vr16[:, g * 128 : (g + 1) * 128],
                                    v4_sb[:, 2 * p + g : 2 * p + g + 1], None,
                                    op0=AL.mult)
                            for scr2, t16 in ((r_scr, r16), (mur_scr, mur16),
                                              (v_scr, vr16)):
                                nc.sync.dma_start(
                                    scr2[2 * p : 2 * p + 2].rearrange(
                                        "g (q f) -> q g f", q=128), t16[:])
                            # broadcast r back; scale x in place (quarters)
                            for q4 in range(4):
                                n0 = q4 * 4096
                                rb = ap.tile([128, 4096], BF16, tag="rb", bufs=1)
                                for g in range(2):
                                    nc.sync.dma_start(
                                        rb[g * 64 : (g + 1) * 64, :],
                                        r_scr[2 * p + g : 2 * p + g + 1,
                                              n0 : n0 + 4096].broadcast_to((64, 4096)))
                                nc.vector.tensor_tensor(
                                    xs[:, n0 : n0 + 4096], xs[:, n0 : n0 + 4096],
                                    rb[:], op=AL.mult)

                    # pin matmuls (lo slot reused across j)
                    with tc.tile_pool(name="ps_b", bufs=2, space="PSUM") as psb, \
                         tc.tile_pool(name="pst", bufs=1) as pst:
                        for j in range(2):
                            lo = pp.tile([70, S], BF16, tag="lo")
                            if j == 0:
                                nc.sync.dma_start(lo[0:64, :], pairs[1][0:64, :])
                            else:
                                nc.sync.dma_start(lo[0:64, :], pairs[0][64:128, :])
                            nc.sync.dma_start(lo[64:67, :], mur_scr[j : j + 3, :])
                            nc.sync.dma_start(lo[67:70, :], v_scr[j : j + 3, :])
                            xp = x1p[j]
                            nc.vector.memset(xp[:, :], 0.0)
                            xpv = xp[:].rearrange("p (h w) -> p h w", h=130)
                            for oh in range(2):
                                for c8 in range(8):
                                    ps = psb.tile([128, 2048], F32, tag="piny")
                                    for c in range(4):
                                        n0 = c8 * 2048 + c * 512
                                        cs = slice(c * 512, (c + 1) * 512)
                                        nc.tensor.matmul(ps[:, cs], w1p_sb[j][oh],
                                                         pairs[j][:, n0 : n0 + 512],
                                                         start=True, stop=False)
                                        nc.tensor.matmul(ps[:, cs], w1lo_sb[j][oh],
                                                         lo[:, n0 : n0 + 512],
                                                         start=False, stop=True)
                                    if oh == 0:
                                        dst = xpv[:, 1 + c8 * 16 : 1 + (c8 + 1) * 16,
                                                  1 : 1 + 128]
                                        nc.scalar.copy(dst, ps[:].rearrange(
                                            "p (h w) -> p h w", h=16))
                                    else:
                                        stg2 = pst.tile([128, 2048], BF16,
                                                        tag="stg2", bufs=2)
                                        nc.scalar.copy(stg2[:], ps[:])
                                        nc.sync.dma_start(
                                            x2d[j][:, c8 * 2048 : (c8 + 1) * 2048],
                                            stg2[:])

                # conv/pool/dyn/gate phase (pair pool closed)
                with tc.tile_pool(name="cp1", bufs=1) as cp1, \
                     tc.tile_pool(name="ps_k", bufs=1, space="PSUM") as psk_p:
                    pc = []
                    for nm, side in (("pa", 66), ("pb", 66), ("pc", 34),
                                     ("pd", 34)):
                        tl = cp1.tile([128, side * side], BF16, tag=nm)
                        nc.vector.memset(tl[:, :], 0.0)
                        pc.append(tl)
                    aw = cp1.tile([128, 8192], BF16, tag="aw")
                    acc = cp1.tile([128, S], BF16, tag="acc")
                    tmp = cp1.tile([128, 2048], BF16, tag="tmp")
                    dump = cp1.tile([128, 1024], BF16, tag="dump")
                    pooled = cp1.tile([128, 2], F32, tag="pooled")
                    kern = cp1.tile([128, 9], F32, tag="kern")

                    for j in range(2):
                        xp = x1p[j]
                        xpv = xp[:].rearrange("p (h w) -> p h w", h=130)
                        x1v = xpv[:, 1:129, 1:129]
                        # avgpool2 (sum; 0.25 folded into layer-0 weights)
                        awv = aw[:].rearrange("p (h w) -> p h w", h=128)
                        xe = x1v.rearrange("p h (w2 two) -> p h w2 two", two=2)
                        nc.vector.tensor_tensor(
                            awv[:], xe[:, :, :, 0], xe[:, :, :, 1], op=AL.add)
                        pav = pc[0][:].rearrange("p (h w) -> p h w", h=66)
                        ae = awv.rearrange("p (h2 two) w -> p h2 two w", two=2)
                        nc.vector.tensor_tensor(
                            pav[:, 1:65, 1:65], ae[:, :, 0, :],
                            ae[:, :, 1, :], op=AL.add)
                        cur = 0
                        for l in range(3):
                            nxt = 1 - cur
                            _dw_taps(nc, pc[nxt][:].rearrange(
                                         "p (h w) -> p h w", h=66),
                                     pc[cur][:].rearrange(
                                         "p (h w) -> p h w", h=66),
                                     64, 64, bw_sb[:, l * 9 : l * 9 + 9],
                                     bb_sb[:, l : l + 1])
                            cur = nxt
                        pbv = pc[cur][:].rearrange("p (h w) -> p h w", h=66)
                        pe = pbv[:, 1:65, 1:65].rearrange(
                            "p h (w2 two) -> p h w2 two", two=2)
                        tmv = tmp[:].rearrange("p (h w) -> p h w", h=64)
                        nc.vector.tensor_tensor(
                            tmv[:, :, 0:32], pe[:, :, :, 0], pe[:, :, :, 1],
                            op=AL.max)
                        te = tmv[:, :, 0:32].rearrange(
                            "p (h2 two) w -> p h2 two w", two=2)
                        pcv = pc[2][:].rearrange("p (h w) -> p h w", h=34)
                        nc.vector.tensor_tensor(
                            pcv[:, 1:33, 1:33], te[:, :, 0, :], te[:, :, 1, :],
                            op=AL.max)
                        cur = 2
                        for l in range(3, 6):
                            nxt = 5 - cur
                            _dw_taps(nc, pc[nxt][:].rearrange(
                                         "p (h w) -> p h w", h=34),
                                     pc[cur][:].rearrange(
                                         "p (h w) -> p h w", h=34),
                                     32, 32, bw_sb[:, l * 9 : l * 9 + 9],
                                     bb_sb[:, l : l + 1])
                            cur = nxt
                        pdv = pc[cur][:].rearrange("p (h w) -> p h w", h=34)
                        nc.scalar.activation(
                            dump[:].rearrange("p (h w) -> p h w", h=32),
                            pdv[:, 1:33, 1:33], ACTF.Copy,
                            scale=1.0 / 1024.0,
                            accum_out=pooled[:, j : j + 1])
                        psk = psk_p.tile([128, 9], F32, tag="psk")
                        for k in range(9):
                            nc.tensor.matmul(psk[:, k : k + 1], tokw_sb[k],
                                             pooled[:, j : j + 1],
                                             start=True, stop=True)
                        nc.scalar.copy(kern[:], psk[:])
                        nc.vector.tensor_tensor(kern[:], kern[:], tokb_sb[:],
                                                op=AL.add)
                        accv = acc[:].rearrange("p (h w) -> p h w", h=128)
                        nc.vector.tensor_scalar(accv[:], x1v, kern[:, 4:5],
                                                dwb_sb[:, :], op0=AL.mult,
                                                op1=AL.add)
                        for ky in range(3):
                            for kx in range(3):
                                if ky == 1 and kx == 1:
                                    continue
                                src2 = xpv[:, ky : ky + 128, kx : kx + 128]
                                nc.vector.scalar_tensor_tensor(
                                    accv[:], src2,
                                    kern[:, ky * 3 + kx : ky * 3 + kx + 1],
                                    accv[:], op0=AL.mult, op1=AL.add)
                        for c8 in range(8):
                            cs = slice(c8 * 2048, (c8 + 1) * 2048)
                            x2t = cp1.tile([128, 2048], BF16, tag="x2c",
                                           bufs=2)
                            nc.sync.dma_start(x2t[:], x2d[j][:, cs])
                            nc.vector.tensor_tensor(acc[:, cs], acc[:, cs],
                                                    x2t[:], op=AL.mult)
                            nc.sync.dma_start(g_out[j][:, cs], acc[:, cs])
    nc.compile()
    return nc


def _build_l2():
    nc = bacc.Bacc("TRN2", target_bir_lowering=False, debug=False, num_devices=8)
    gh = nc.dram_tensor("gh", [4, 128, S], BF16, kind="ExternalInput")
    xres = nc.dram_tensor("xres", [2, 64, S], F32, kind="ExternalInput")
    w2 = nc.dram_tensor("w2", [3, 128, 64], BF16, kind="ExternalInput")
    z_out = [nc.dram_tensor(f"z{j}", [64, S], F32, kind="ExternalOutput")
             for j in range(2)]
    with tile.TileContext(nc, pool_alloc_mode="queue") as tc:
        with tc.tile_pool(name="wp", bufs=1) as wp, \
             tc.tile_pool(name="ps", bufs=2, space="PSUM") as psp:
            w2_sb = []
            for tau in range(3):
                tw2 = wp.tile([128, 64], BF16, tag=f"w2{tau}")
                nc.sync.dma_start(tw2[:], w2[tau])
                w2_sb.append(tw2)
            gsb = []
            for s in range(4):
                g = wp.tile([128, S], BF16, tag=f"g{s}")
                nc.sync.dma_start(g[:], gh[s])
                gsb.append(g)
            for j in range(2):
                for c8 in range(8):
                    ps = psp.tile([64, 2048], F32, tag="z")
                    for c in range(4):
                        n0 = c8 * 2048 + c * 512
                        cs = slice(c * 512, (c + 1) * 512)
                        for tau in range(3):
                            nc.tensor.matmul(ps[:, cs], w2_sb[tau],
                                             gsb[j + tau][:, n0 : n0 + 512],
                                             start=(tau == 0), stop=(tau == 2))
                    xrt = wp.tile([64, 2048], F32, tag="xr", bufs=2)
                    nc.sync.dma_start(
                        xrt[:], xres[j][:, c8 * 2048 : (c8 + 1) * 2048])
                    ot = wp.tile([64, 2048], F32, tag="ot", bufs=2)
                    nc.vector.tensor_tensor(ot[:], ps[:], xrt[:], op=AL.add)
                    nc.sync.dma_start(z_out[j][:, c8 * 2048 : (c8 + 1) * 2048],
                                      ot[:])
    nc.compile()
    return nc


def _prep_weights(ln_w, ln_b, pin_w, pout_w, b1_w, b1_b, b2_w, b2_b, tok_w,
                  tok_b, dw_bias):
    pw = np.asarray(pin_w)[:, :, :, 0, 0].astype(np.float64)  # (256, 64, 3)
    lnw = np.asarray(ln_w).astype(np.float64)
    lnb = np.asarray(ln_b).astype(np.float64)
    W1 = [(pw[:, :, t] * lnw[None, :]).T for t in range(3)]  # (64, 256) each
    s1 = [(pw[:, :, t] * lnw[None, :]).sum(1) for t in range(3)]  # (256,)
    bias1 = [pw[:, :, t] @ lnb for t in range(3)]  # (256,)
    w1p = np.zeros((2, 2, 128, 128), np.float32)
    w1lo = np.zeros((2, 2, 70, 128), np.float32)
    for j in range(2):
        tA, tB = (0, 1) if j == 0 else (1, 2)
        tlo = 2 if j == 0 else 0
        for oh in range(2):
            ohs = slice(oh * 128, (oh + 1) * 128)
            w1p[j, oh, 0:64] = W1[tA][:, ohs]
            w1p[j, oh, 64:128] = W1[tB][:, ohs]
            w1lo[j, oh, 0:64] = W1[tlo][:, ohs]
            for t in range(3):
                w1lo[j, oh, 64 + t] = -s1[t][ohs]
                w1lo[j, oh, 67 + t] = bias1[t][ohs]
    bw = np.zeros((128, 54), np.float32)
    bb = np.zeros((128, 6), np.float32)
    b1w = np.asarray(b1_w)[:, :, 0]  # (3, 128, 3, 3)
    b2w = np.asarray(b2_w)[:, :, 0]
    for l in range(3):
        bw[:, l * 9 : l * 9 + 9] = b1w[l].reshape(128, 9)
        bw[:, (l + 3) * 9 : (l + 3) * 9 + 9] = b2w[l].reshape(128, 9)
        bb[:, l] = np.asarray(b1_b)[l]
        bb[:, l + 3] = np.asarray(b2_b)[l]
    bw[:, 0:9] *= 0.25  # avgpool mean folded into layer-0 taps
    tokw = np.zeros((9, 128, 128), np.float32)
    tw = np.asarray(tok_w)  # (1152, 128)
    for k in range(9):
        tokw[k] = tw[k::9, :].T  # [h, c] = tok_w[c*9+k, h]
    tokb = np.asarray(tok_b).reshape(128, 9).astype(np.float32)
    w2 = np.zeros((3, 128, 64), np.float32)
    pow_ = np.asarray(pout_w)[:, :, :, 0, 0]  # (64, 128, 3)
    for t in range(3):
        w2[t] = pow_[:, :, t].T
    dwb = np.asarray(dw_bias).reshape(128, 1).astype(np.float32)
    return (w1p.astype(BF), w1lo.astype(BF), bw, bb, tokw, tokb,
            w2.astype(BF), dwb)


def kernel(x, ln_w, ln_b, pin_w, pout_w, b1_w, b1_b, b2_w, b2_b, tok_w, tok_b,
           dw_bias):
    x = np.asarray(x)
    (w1p, w1lo, bw, bb, tokw, tokb, w2, dwb) = _prep_weights(
        ln_w, ln_b, pin_w, pout_w, b1_w, b1_b, b2_w, b2_b, tok_w, tok_b,
        dw_bias)
    if "l1" not in _cache:
        _cache["l1"] = _build_l1()
    if "l2" not in _cache:
        _cache["l2"] = _build_l2()

    xbf = x.astype(BF)  # (B, T, C, H, W)
    in_maps1 = []
    for i in range(8):
        b, t0 = i // 4, 2 * (i % 4)
        xh = np.zeros((4, C, S), BF)
        v4 = np.zeros((4,), np.float32)
        for k in range(4):
            t = t0 - 1 + k
            if 0 <= t < T:
                xh[k] = xbf[b, t].reshape(C, S)
                v4[k] = 1.0
        in_maps1.append({
            "xh": xh, "v4b": np.broadcast_to(v4, (128, 4)).copy(),
            "w1p": w1p, "w1lo": w1lo, "bw": bw, "bb": bb, "tokw": tokw,
            "tokb": tokb, "dwb": dwb})
    r1 = run_bass_kernel_spmd(_cache["l1"], in_maps1, core_ids=list(range(8)),
                              trace=TRACE)
    PROF["l1"] = r1

    gated = np.zeros((B, T, 128, S), BF)
    for i in range(8):
        b, t0 = i // 4, 2 * (i % 4)
        gated[b, t0] = r1.results[i]["g0"]
        gated[b, t0 + 1] = r1.results[i]["g1"]

    in_maps2 = []
    for i in range(8):
        b, t0 = i // 4, 2 * (i % 4)
        gh = np.zeros((4, 128, S), BF)
        for k in range(4):
            t = t0 - 1 + k
            if 0 <= t < T:
                gh[k] = gated[b, t]
        xres = x[b, t0 : t0 + 2].reshape(2, C, S).astype(np.float32)
        in_maps2.append({"gh": gh, "xres": xres, "w2": w2})
    r2 = run_bass_kernel_spmd(_cache["l2"], in_maps2, core_ids=list(range(8)),
                              trace=TRACE)
    PROF["l2"] = r2

    out = np.zeros((B, T, C, H, W), np.float32)
    for i in range(8):
        b, t0 = i // 4, 2 * (i % 4)
        out[b, t0] = r2.results[i]["z0"].reshape(C, H, W)
        out[b, t0 + 1] = r2.results[i]["z1"].reshape(C, H, W)
    return out



# revision 8
# speedup vs baseline: 1.8546x; 1.8546x over previous
"""Trainium2 Bass kernel for nn_CWGDN (dense_cnn): LN -> temporal pin conv ->
dynamic depthwise conv (w/ pooled kernel-generator branch) -> gate -> temporal
pout conv + residual.

Sharding: 16 (b,t) instances over 8 cores (2 each). Two SPMD launches:
  L1: per-core slices [t0-1, t0+3) of x -> gated(t0), gated(t0+1)   (bf16)
  L2: host reshards gated by H-slab (pixel parallel) -> pout conv + residual

Depthwise 3x3 convs (B1 tower + dynamic conv) run on the TensorEngine as
diagonal-stationary matmuls accumulating 9 shifted taps in PSUM; the B2 tower
is folded on the host into a per-channel 32x32 mask K with pooled =
<K, maxpool_out> + const (exact: B2 is linear after the maxpool).
LayerNorm is folded into the pin matmul (pre-scale by rsqrt(var+eps) plus
rank-1 correction rows).
"""
import sys

sys.path.insert(0, "/opt/trn_rl_repo")

import numpy as np
import ml_dtypes

import concourse.bass as bass
import concourse.tile as tile
from concourse import bacc, mybir
from concourse.bass_utils import run_bass_kernel_spmd

BF = ml_dtypes.bfloat16
F32 = mybir.dt.float32
BF16 = mybir.dt.bfloat16
AL = mybir.AluOpType
ACTF = mybir.ActivationFunctionType

B, T, C, H, W = 2, 8, 64, 128, 128
HID = 128
S = H * W  # 16384
K = 3
EPS = 1e-5

_cache = {}
TRACE = False
PROF = {}




def _build_l1():
    nc = bacc.Bacc("TRN2", target_bir_lowering=False, debug=False, num_devices=8)
    xh = nc.dram_tensor("xh", [4, C, S], BF16, kind="ExternalInput")
    v4b = nc.dram_tensor("v4b", [128, 4], F32, kind="ExternalInput")
    w1p = nc.dram_tensor("w1p", [2, 2, 128, 128], BF16, kind="ExternalInput")
    w1lo = nc.dram_tensor("w1lo", [2, 2, 70, 128], BF16, kind="ExternalInput")
    b1d = nc.dram_tensor("b1d", [128, 31 * 128], BF16, kind="ExternalInput")
    bb = nc.dram_tensor("bb", [128, 3], F32, kind="ExternalInput")
    i128 = nc.dram_tensor("i128", [128, 128], BF16, kind="ExternalInput")
    kmask = nc.dram_tensor("kmask", [128, 1024], BF16, kind="ExternalInput")
    tokw = nc.dram_tensor("tokw", [9, 128, 128], F32, kind="ExternalInput")
    tokb = nc.dram_tensor("tokb", [128, 9], F32, kind="ExternalInput")
    dwb = nc.dram_tensor("dwb", [128, 1], F32, kind="ExternalInput")
    g_out = [nc.dram_tensor(f"g{j}", [128, S], BF16, kind="ExternalOutput")
             for j in range(2)]
    # internal DRAM scratch
    scr_s = nc.dram_tensor("scr_s", [2, 2, S], F32)
    scr_q = nc.dram_tensor("scr_q", [2, 2, S], F32)
    r_scr = nc.dram_tensor("r_scr", [4, S], BF16)
    mur_scr = nc.dram_tensor("mur_scr", [4, S], BF16)
    v_scr = nc.dram_tensor("v_scr", [4, S], BF16)
    x2d = nc.dram_tensor("x2d", [2, 128, S], BF16)

    with tile.TileContext(nc, pool_alloc_mode="queue") as tc:
        with tc.tile_pool(name="wp", bufs=1) as wp:
            w1p_sb, w1lo_sb = [], []
            for j in range(2):
                w1p_sb.append([])
                w1lo_sb.append([])
                for oh in range(2):
                    tp = wp.tile([128, 128], BF16, tag=f"w1p{j}{oh}")
                    nc.sync.dma_start(tp[:], w1p[j, oh])
                    w1p_sb[j].append(tp)
                    tl = wp.tile([70, 128], BF16, tag=f"w1lo{j}{oh}")
                    nc.sync.dma_start(tl[:], w1lo[j, oh])
                    w1lo_sb[j].append(tl)
            bb_sb = wp.tile([128, 3], F32, tag="bb")
            nc.sync.dma_start(bb_sb[:], bb[:])
            tokw_sb = []
            for k in range(9):
                tk = wp.tile([128, 128], F32, tag=f"tokw{k}")
                nc.sync.dma_start(tk[:], tokw[k])
                tokw_sb.append(tk)
            tokb_sb = wp.tile([128, 9], F32, tag="tokb")
            nc.sync.dma_start(tokb_sb[:], tokb[:])
            dwb_sb = wp.tile([128, 1], F32, tag="dwb")
            nc.sync.dma_start(dwb_sb[:], dwb[:])
            v4_sb = wp.tile([128, 4], F32, tag="v4")
            nc.sync.dma_start(v4_sb[:], v4b[:])
            i2 = wp.tile([128, 2], BF16, tag="i2")
            nc.gpsimd.memset(i2[:, :], 0.0)
            nc.gpsimd.memset(i2[0:64, 0:1], 1.0)
            nc.gpsimd.memset(i2[64:128, 1:2], 1.0)
            eps_t = wp.tile([128, 1], F32, tag="eps")
            nc.gpsimd.memset(eps_t[:, :], EPS)

            with tc.tile_pool(name="cp0", bufs=1) as cp0:
                x1p = [cp0.tile([128, 130 * 130 + 8], BF16, tag=f"x1_{j}",
                                name=f"x1t{j}") for j in range(2)]
                pairs = []
                with tc.tile_pool(name="pp", bufs=1) as pp:
                    with tc.tile_pool(name="ap", bufs=1) as ap, \
                         tc.tile_pool(name="ps_a", bufs=1, space="PSUM") as psa:
                        for p in range(2):
                            xs = pp.tile([128, S], BF16, tag=f"pair{p}")
                            pairs.append(xs)
                            nc.sync.dma_start(
                                xs[:],
                                xh[2 * p : 2 * p + 2].rearrange("s c f -> (s c) f"))
                            # stats: per-pixel channel sums of x and x^2
                            for q in range(8):
                                n0 = q * 2048
                                sq = ap.tile([128, 2048], BF16, tag="sq", bufs=2)
                                nc.scalar.activation(sq[:], xs[:, n0 : n0 + 2048],
                                                     ACTF.Square)
                                psS = psa.tile([2, 2048], F32, tag="stS")
                                psQ = psa.tile([2, 2048], F32, tag="stQ")
                                for c in range(4):
                                    cs = slice(c * 512, (c + 1) * 512)
                                    nc.tensor.matmul(psS[:, cs], i2[:],
                                                     xs[:, n0 + c * 512 : n0 + (c + 1) * 512],
                                                     start=True, stop=True)
                                    nc.tensor.matmul(psQ[:, cs], i2[:],
                                                     sq[:, cs],
                                                     start=True, stop=True)
                                stgS = ap.tile([2, 2048], F32, tag="stgS", bufs=1)
                                nc.scalar.copy(stgS[:], psS[:])
                                nc.sync.dma_start(scr_s[p, :, n0 : n0 + 2048], stgS[:])
                                stgQ = ap.tile([2, 2048], F32, tag="stgQ", bufs=1)
                                nc.vector.tensor_copy(stgQ[:], psQ[:])
                                nc.sync.dma_start(scr_q[p, :, n0 : n0 + 2048], stgQ[:])
                            # r / mur in pixel-spread layout (128, 256)
                            sp_s = ap.tile([128, 256], F32, tag="sp_s")
                            nc.sync.dma_start(
                                sp_s[:], scr_s[p].rearrange("g (q f) -> q g f", q=128))
                            sp_q = ap.tile([128, 256], F32, tag="sp_q")
                            nc.sync.dma_start(
                                sp_q[:], scr_q[p].rearrange("g (q f) -> q g f", q=128))
                            mu = ap.tile([128, 256], F32, tag="mu")
                            nc.scalar.mul(mu[:], sp_s[:], 1.0 / 64.0)
                            msq = ap.tile([128, 256], F32, tag="msq")
                            nc.vector.tensor_tensor(msq[:], mu[:], mu[:], op=AL.mult)
                            nc.vector.scalar_tensor_tensor(
                                sp_q[:], sp_q[:], 1.0 / 64.0, msq[:],
                                op0=AL.mult, op1=AL.subtract)  # var -> sp_q
                            nc.scalar.activation(sp_s[:], sp_q[:], ACTF.Sqrt,
                                                 bias=eps_t[:, :])  # std -> sp_s
                            nc.vector.reciprocal(sp_q[:], sp_s[:])  # r -> sp_q
                            for g in range(2):
                                nc.vector.tensor_scalar(
                                    sp_q[:, g * 128 : (g + 1) * 128],
                                    sp_q[:, g * 128 : (g + 1) * 128],
                                    v4_sb[:, 2 * p + g : 2 * p + g + 1], None,
                                    op0=AL.mult)
                            r16 = ap.tile([128, 256], BF16, tag="r16")
                            nc.vector.tensor_copy(r16[:], sp_q[:])
                            nc.vector.tensor_tensor(mu[:], mu[:], sp_q[:],
                                                    op=AL.mult)  # mur -> mu
                            mur16 = ap.tile([128, 256], BF16, tag="mur16")
                            nc.vector.tensor_copy(mur16[:], mu[:])
                            vr16 = ap.tile([128, 256], BF16, tag="vr16")
                            nc.gpsimd.memset(vr16[:, :], 1.0)
                            for g in range(2):
                                nc.vector.tensor_scalar(
                                    vr16[:, g * 128 : (g + 1) * 128],
                                    vr16[:, g * 128 : (g + 1) * 128],
                                    v4_sb[:, 2 * p + g : 2 * p + g + 1], None,
                                    op0=AL.mult)
                            for scr2, t16 in ((r_scr, r16), (mur_scr, mur16),
                                              (v_scr, vr16)):
                                nc.sync.dma_start(
                                    scr2[2 * p : 2 * p + 2].rearrange(
                                        "g (q f) -> q g f", q=128), t16[:])
                            # broadcast r back; scale x in place (quarters)
                            for q4 in range(4):
                                n0 = q4 * 4096
                                rb = ap.tile([128, 4096], BF16, tag="rb", bufs=1)
                                for g in range(2):
                                    nc.sync.dma_start(
                                        rb[g * 64 : (g + 1) * 64, :],
                                        r_scr[2 * p + g : 2 * p + g + 1,
                                              n0 : n0 + 4096].broadcast_to((64, 4096)))
                                nc.vector.tensor_tensor(
                                    xs[:, n0 : n0 + 4096], xs[:, n0 : n0 + 4096],
                                    rb[:], op=AL.mult)

                    # pin matmuls (lo slot reused across j)
                    with tc.tile_pool(name="ps_b", bufs=2, space="PSUM") as psb, \
                         tc.tile_pool(name="pst", bufs=1) as pst:
                        for j in range(2):
                            lo = pp.tile([70, S], BF16, tag="lo")
                            if j == 0:
                                nc.sync.dma_start(lo[0:64, :], pairs[1][0:64, :])
                            else:
                                nc.sync.dma_start(lo[0:64, :], pairs[0][64:128, :])
                            nc.sync.dma_start(lo[64:67, :], mur_scr[j : j + 3, :])
                            nc.sync.dma_start(lo[67:70, :], v_scr[j : j + 3, :])
                            xp = x1p[j]
                            nc.vector.memset(xp[:, :], 0.0)
                            xpv = xp[:, 0 : 130 * 130].rearrange(
                                "p (h w) -> p h w", h=130)
                            for oh in range(2):
                                for c8 in range(8):
                                    ps = psb.tile([128, 2048], F32, tag="piny")
                                    for c in range(4):
                                        n0 = c8 * 2048 + c * 512
                                        cs = slice(c * 512, (c + 1) * 512)
                                        nc.tensor.matmul(ps[:, cs], w1p_sb[j][oh],
                                                         pairs[j][:, n0 : n0 + 512],
                                                         start=True, stop=False)
                                        nc.tensor.matmul(ps[:, cs], w1lo_sb[j][oh],
                                                         lo[:, n0 : n0 + 512],
                                                         start=False, stop=True)
                                    if oh == 0:
                                        dst = xpv[:, 1 + c8 * 16 : 1 + (c8 + 1) * 16,
                                                  1 : 1 + 128]
                                        nc.scalar.copy(dst, ps[:].rearrange(
                                            "p (h w) -> p h w", h=16))
                                    else:
                                        stg2 = pst.tile([128, 2048], BF16,
                                                        tag="stg2", bufs=2)
                                        nc.scalar.copy(stg2[:], ps[:])
                                        nc.sync.dma_start(
                                            x2d[j][:, c8 * 2048 : (c8 + 1) * 2048],
                                            stg2[:])

                # conv/pool/dyn/gate phase (pair pool closed)
                with tc.tile_pool(name="cp1", bufs=1) as cp1, \
                     tc.tile_pool(name="dgp", bufs=2) as dgp, \
                     tc.tile_pool(name="ps_k", bufs=1, space="PSUM") as psk_p, \
                     tc.tile_pool(name="ps_c", bufs=3, space="PSUM") as psc:
                    b1d_sb = cp1.tile([128, 31 * 128], BF16, tag="b1d")
                    nc.sync.dma_start(b1d_sb[:], b1d[:])
                    ones_sb = cp1.tile([128, 512], BF16, tag="ones")
                    nc.gpsimd.memset(ones_sb[:, :], 1.0)
                    i128_sb = cp1.tile([128, 128], BF16, tag="i128")
                    nc.sync.dma_start(i128_sb[:], i128[:])
                    kmask_sb = cp1.tile([128, 1024], BF16, tag="kmask")
                    nc.sync.dma_start(kmask_sb[:], kmask[:])
                    pc = []
                    for nm in ("pa", "pb"):
                        tl = cp1.tile([128, 66 * 66 + 4], BF16, tag=nm)
                        nc.vector.memset(tl[:, :], 0.0)
                        pc.append(tl)
                    aw = cp1.tile([128, 8192], BF16, tag="aw")
                    acc = cp1.tile([128, S], BF16, tag="acc")
                    tmp = cp1.tile([128, 2048], BF16, tag="tmp")
                    qt = cp1.tile([128, 1024], BF16, tag="qt")
                    qscr = cp1.tile([128, 1024], BF16, tag="qscr")
                    pooled = cp1.tile([128, 2], F32, tag="pooled")
                    kern = cp1.tile([128, 9], F32, tag="kern")

                    for j in range(2):
                        xp = x1p[j]
                        xpv = xp[:, 0 : 130 * 130].rearrange(
                            "p (h w) -> p h w", h=130)
                        x1v = xpv[:, 1:129, 1:129]
                        # avgpool2 (sum; 0.25 folded into layer-0 weights)
                        awv = aw[:].rearrange("p (h w) -> p h w", h=128)
                        xe = x1v.rearrange("p h (w2 two) -> p h w2 two", two=2)
                        nc.vector.tensor_tensor(
                            awv[:], xe[:, :, :, 0], xe[:, :, :, 1], op=AL.add)
                        pav = pc[0][:, 0 : 66 * 66].rearrange(
                            "p (h w) -> p h w", h=66)
                        ae = awv.rearrange("p (h2 two) w -> p h2 two w", two=2)
                        nc.vector.tensor_tensor(
                            pav[:, 1:65, 1:65], ae[:, :, 0, :],
                            ae[:, :, 1, :], op=AL.add)
                        # B1 tower: 3 diag-stationary PE convs, 64x64
                        cur = 0
                        for l in range(3):
                            nxt = 1 - cur
                            sflat = pc[cur]
                            dv = pc[nxt][:, 0 : 66 * 66].rearrange(
                                "p (h w) -> p h w", h=66)
                            for y0 in range(0, 64, 7):
                                nr = min(7, 64 - y0)
                                N = nr * 66
                                f0 = (1 + y0) * 66 + 1
                                psb1 = psc.tile([128, 512], F32, tag="psb1")
                                for ky in range(3):
                                    for kx in range(3):
                                        t = ky * 3 + kx
                                        dcol = (l * 9 + t) * 128
                                        o = f0 + (ky - 1) * 66 + (kx - 1)
                                        nc.tensor.matmul(
                                            psb1[:, 0:N],
                                            b1d_sb[:, dcol : dcol + 128],
                                            sflat[:, o : o + N],
                                            start=(t == 0), stop=False)
                                bcol = (27 + l) * 128
                                nc.tensor.matmul(
                                    psb1[:, 0:N],
                                    b1d_sb[:, bcol : bcol + 128],
                                    ones_sb[:, 0:N], start=False, stop=True)
                                nc.scalar.copy(
                                    dv[:, 1 + y0 : 1 + y0 + nr, 1:65],
                                    psb1[:, 0:N].rearrange(
                                        "p (h w) -> p h w", h=nr)[:, :, 0:64])
                            cur = nxt
                        # maxpool 2x2 -> compact q (128, 1024)
                        pbv = pc[cur][:, 0 : 66 * 66].rearrange(
                            "p (h w) -> p h w", h=66)
                        pe = pbv[:, 1:65, 1:65].rearrange(
                            "p h (w2 two) -> p h w2 two", two=2)
                        tmv = tmp[:].rearrange("p (h w) -> p h w", h=64)
                        nc.vector.tensor_tensor(
                            tmv[:, :, 0:32], pe[:, :, :, 0], pe[:, :, :, 1],
                            op=AL.max)
                        te = tmv[:, :, 0:32].rearrange(
                            "p (h2 two) w -> p h2 two w", two=2)
                        qv = qt[:].rearrange("p (h w) -> p h w", h=32)
                        nc.vector.tensor_tensor(
                            qv[:, :, :], te[:, :, 0, :], te[:, :, 1, :],
                            op=AL.max)
                        # pooled = sum(q * kmask); kconst folded into tokb
                        nc.vector.tensor_tensor(qscr[:], qt[:], kmask_sb[:],
                                                op=AL.mult)
                        nc.scalar.activation(qt[:], qscr[:], ACTF.Copy,
                                             accum_out=pooled[:, j : j + 1])
                        psk = psk_p.tile([128, 9], F32, tag="psk")
                        for k in range(9):
                            nc.tensor.matmul(psk[:, k : k + 1], tokw_sb[k],
                                             pooled[:, j : j + 1],
                                             start=True, stop=True)
                        nc.scalar.copy(kern[:], psk[:])
                        nc.vector.tensor_tensor(kern[:], kern[:], tokb_sb[:],
                                                op=AL.add)
                        # dynamic dw conv: 9 diag-stationary taps on PE
                        dg = [dgp.tile([128, 128], BF16, tag=f"dg{k}",
                                       name=f"dgt{k}") for k in range(9)]
                        for k in range(9):
                            nc.vector.tensor_scalar(dg[k][:], i128_sb[:],
                                                    kern[:, k : k + 1], None,
                                                    op0=AL.mult)
                        for y0 in range(0, 128, 3):
                            nr = min(3, 128 - y0)
                            N = nr * 130
                            f0 = (1 + y0) * 130 + 1
                            psd = psc.tile([128, 512], F32, tag="psd")
                            for ky in range(3):
                                for kx in range(3):
                                    t = ky * 3 + kx
                                    o = f0 + (ky - 1) * 130 + (kx - 1)
                                    nc.tensor.matmul(
                                        psd[:, 0:N], dg[t][:],
                                        xp[:, o : o + N],
                                        start=(t == 0), stop=False)
                            nc.tensor.matmul(
                                psd[:, 0:N], b1d_sb[:, 30 * 128 : 31 * 128],
                                ones_sb[:, 0:N], start=False, stop=True)
                            nc.scalar.copy(
                                acc[:, y0 * 128 : (y0 + nr) * 128],
                                psd[:, 0:N].rearrange(
                                    "p (h w) -> p h w", h=nr)[:, :, 0:128])
                        for c8 in range(8):
                            cs = slice(c8 * 2048, (c8 + 1) * 2048)
                            x2t = cp1.tile([128, 2048], BF16, tag="x2c",
                                           bufs=2)
                            nc.sync.dma_start(x2t[:], x2d[j][:, cs])
                            nc.vector.tensor_tensor(acc[:, cs], acc[:, cs],
                                                    x2t[:], op=AL.mult)
                            nc.sync.dma_start(g_out[j][:, cs], acc[:, cs])
    nc.compile()
    return nc


def _build_l2():
    # pixel-sharded: each core handles a 16-row H-slab (2048 px) of ALL 16
    # (b,t) instances; t-pairs stacked on partitions (z(t0) ch 0-63,
    # z(t0+1) ch 64-127); residual rides the matmul via identity stationary.
    nc = bacc.Bacc("TRN2", target_bir_lowering=False, debug=False, num_devices=8)
    SL = 2048
    gh = nc.dram_tensor("gh", [16, 128, SL], BF16, kind="ExternalInput")
    xr = nc.dram_tensor("xr", [8, 128, SL], BF16, kind="ExternalInput")
    w2s = nc.dram_tensor("w2s", [5, 128, 128], BF16, kind="ExternalInput")
    zz = nc.dram_tensor("zz", [8, 128, SL], BF16, kind="ExternalOutput")
    with tile.TileContext(nc, pool_alloc_mode="queue") as tc:
        with tc.tile_pool(name="wp", bufs=1) as wp, \
             tc.tile_pool(name="ps", bufs=3, space="PSUM") as psp:
            w2_sb = []
            for k in range(5):
                tw2 = wp.tile([128, 128], BF16, tag=f"w2{k}")
                nc.sync.dma_start(tw2[:], w2s[k])
                w2_sb.append(tw2)
            gsb = []
            for s in range(16):
                g = wp.tile([128, SL], BF16, tag=f"g{s}")
                nc.sync.dma_start(g[:], gh[s])
                gsb.append(g)
            xsb = []
            for tp in range(8):
                xt = wp.tile([128, SL], BF16, tag=f"x{tp}")
                nc.sync.dma_start(xt[:], xr[tp])
                xsb.append(xt)
            for tp in range(8):
                b, t0 = tp // 4, 2 * (tp % 4)
                ot = wp.tile([128, SL], BF16, tag="ot", bufs=2)
                for blk in range(4):
                    cs = slice(blk * 512, (blk + 1) * 512)
                    ps = psp.tile([128, 512], F32, tag="z")
                    passes = [(w2_sb[k], gsb[b * 8 + t0 - 1 + k])
                              for k in range(4) if 0 <= t0 - 1 + k < 8]
                    passes.append((w2_sb[4], xsb[tp]))
                    for i, (st, mvt) in enumerate(passes):
                        nc.tensor.matmul(ps[:], st[:], mvt[:, cs],
                                         start=(i == 0),
                                         stop=(i == len(passes) - 1))
                    nc.scalar.copy(ot[:, cs], ps[:])
                nc.sync.dma_start(zz[tp], ot[:])
    nc.compile()
    return nc


def _corr3(x, w):
    # x: (C,H,W), w: (C,3,3); zero-padded 'same' correlation (matches lax conv)
    Cc, Hh, Ww = x.shape
    xp = np.zeros((Cc, Hh + 2, Ww + 2), np.float64)
    xp[:, 1:-1, 1:-1] = x
    y = np.zeros((Cc, Hh, Ww), np.float64)
    for a in range(3):
        for b in range(3):
            y += w[:, a, b, None, None] * xp[:, a : a + Hh, b : b + Ww]
    return y


def _prep_weights(ln_w, ln_b, pin_w, pout_w, b1_w, b1_b, b2_w, b2_b, tok_w,
                  tok_b, dw_bias):
    pw = np.asarray(pin_w)[:, :, :, 0, 0].astype(np.float64)  # (256, 64, 3)
    lnw = np.asarray(ln_w).astype(np.float64)
    lnb = np.asarray(ln_b).astype(np.float64)
    W1 = [(pw[:, :, t] * lnw[None, :]).T for t in range(3)]  # (64, 256) each
    s1 = [(pw[:, :, t] * lnw[None, :]).sum(1) for t in range(3)]  # (256,)
    bias1 = [pw[:, :, t] @ lnb for t in range(3)]  # (256,)
    w1p = np.zeros((2, 2, 128, 128), np.float32)
    w1lo = np.zeros((2, 2, 70, 128), np.float32)
    for j in range(2):
        tA, tB = (0, 1) if j == 0 else (1, 2)
        tlo = 2 if j == 0 else 0
        for oh in range(2):
            ohs = slice(oh * 128, (oh + 1) * 128)
            w1p[j, oh, 0:64] = W1[tA][:, ohs]
            w1p[j, oh, 64:128] = W1[tB][:, ohs]
            w1lo[j, oh, 0:64] = W1[tlo][:, ohs]
            for t in range(3):
                w1lo[j, oh, 64 + t] = -s1[t][ohs]
                w1lo[j, oh, 67 + t] = bias1[t][ohs]
    # B1 tower diag stationaries: b1d[p, (l*9+t)*128 + f] = diag(b1w[l,:,ky,kx])
    b1w = np.asarray(b1_w)[:, :, 0].astype(np.float64)  # (3, 128, 3, 3)
    b1w = b1w.copy()
    b1w[0] *= 0.25  # avgpool mean folded into layer-0 taps
    b1dflat = np.zeros((128, 31 * 128), np.float32)
    ar = np.arange(128)
    for l in range(3):
        for t in range(9):
            ky, kx = t // 3, t % 3
            col = (l * 9 + t) * 128
            b1dflat[ar, col + ar] = b1w[l, :, ky, kx]
        b1dflat[ar, (27 + l) * 128 + ar] = np.asarray(b1_b)[l]
    b1dflat[ar, 30 * 128 + ar] = np.asarray(dw_bias)
    bb = np.asarray(b1_b).T.astype(np.float32).copy()  # (128, 3)
    # B2 tower linearized: pooled = <K, q> + const
    b2w = np.asarray(b2_w)[:, :, 0].astype(np.float64)  # (3, 128, 3, 3)
    b2b = np.asarray(b2_b).astype(np.float64)  # (3, 128)
    U = np.ones((128, 32, 32)) / 1024.0
    Km = _corr3(_corr3(_corr3(U, b2w[2][:, ::-1, ::-1]),
                       b2w[1][:, ::-1, ::-1]), b2w[0][:, ::-1, ::-1])
    t = _corr3(np.zeros((128, 32, 32)), b2w[0]) + b2b[0][:, None, None]
    t = _corr3(t, b2w[1]) + b2b[1][:, None, None]
    t = _corr3(t, b2w[2]) + b2b[2][:, None, None]
    kconst = t.mean(axis=(1, 2)).astype(np.float32).reshape(128, 1)
    kmask = Km.reshape(128, 1024).astype(np.float32)
    tokw = np.zeros((9, 128, 128), np.float32)
    tw = np.asarray(tok_w)  # (1152, 128)
    for k in range(9):
        tokw[k] = tw[k::9, :].T  # [h, c] = tok_w[c*9+k, h]
    tokb = np.asarray(tok_b).reshape(128, 9).astype(np.float64)
    for k in range(9):
        tokb[:, k] += tokw[k].T.astype(np.float64) @ kconst[:, 0].astype(np.float64)
    tokb = tokb.astype(np.float32)
    # L2 stationaries: S_k maps gated slice s=t0-1+k to z-pair (t0, t0+1)
    pow_ = np.asarray(pout_w)[:, :, :, 0, 0]  # (64, 128, 3)
    w2t = [pow_[:, :, t].T for t in range(3)]  # (128, 64) each: in x out
    w2s = np.zeros((5, 128, 128), np.float32)
    for k in range(4):
        if k < 3:
            w2s[k, :, 0:64] = w2t[k]
        if k >= 1:
            w2s[k, :, 64:128] = w2t[k - 1]
    w2s[4] = np.eye(128)
    dwb = np.asarray(dw_bias).reshape(128, 1).astype(np.float32)
    i128 = np.eye(128, dtype=np.float32)
    return (w1p.astype(BF), w1lo.astype(BF), b1dflat.astype(BF), bb,
            kmask.astype(BF), tokw, tokb, w2s.astype(BF), dwb,
            i128.astype(BF))


def kernel(x, ln_w, ln_b, pin_w, pout_w, b1_w, b1_b, b2_w, b2_b, tok_w, tok_b,
           dw_bias):
    x = np.asarray(x)
    (w1p, w1lo, b1dflat, bb, kmask, tokw, tokb, w2s, dwb,
     i128) = _prep_weights(ln_w, ln_b, pin_w, pout_w, b1_w, b1_b, b2_w, b2_b,
                           tok_w, tok_b, dw_bias)
    if "l1" not in _cache:
        _cache["l1"] = _build_l1()
    if "l2" not in _cache:
        _cache["l2"] = _build_l2()

    xbf = x.astype(BF)  # (B, T, C, H, W)
    in_maps1 = []
    for i in range(8):
        b, t0 = i // 4, 2 * (i % 4)
        xh = np.zeros((4, C, S), BF)
        v4 = np.zeros((4,), np.float32)
        for k in range(4):
            t = t0 - 1 + k
            if 0 <= t < T:
                xh[k] = xbf[b, t].reshape(C, S)
                v4[k] = 1.0
        in_maps1.append({
            "xh": xh, "v4b": np.broadcast_to(v4, (128, 4)).copy(),
            "w1p": w1p, "w1lo": w1lo, "b1d": b1dflat, "bb": bb, "i128": i128,
            "kmask": kmask, "tokw": tokw, "tokb": tokb, "dwb": dwb})
    r1 = run_bass_kernel_spmd(_cache["l1"], in_maps1, core_ids=list(range(8)),
                              trace=TRACE)
    PROF["l1"] = r1

    gated = np.zeros((B, T, 128, S), BF)
    for i in range(8):
        b, t0 = i // 4, 2 * (i % 4)
        gated[b, t0] = r1.results[i]["g0"]
        gated[b, t0 + 1] = r1.results[i]["g1"]

    SL = 2048
    in_maps2 = []
    for i in range(8):
        sl = slice(SL * i, SL * (i + 1))
        gh = gated[:, :, :, sl].reshape(16, 128, SL)
        xr = np.zeros((8, 128, SL), BF)
        for tp in range(8):
            b, t0 = tp // 4, 2 * (tp % 4)
            xr[tp, 0:64] = xbf[b, t0].reshape(C, S)[:, sl]
            xr[tp, 64:128] = xbf[b, t0 + 1].reshape(C, S)[:, sl]
        in_maps2.append({"gh": np.ascontiguousarray(gh), "xr": xr, "w2s": w2s})
    r2 = run_bass_kernel_spmd(_cache["l2"], in_maps2, core_ids=list(range(8)),
                              trace=TRACE)
    PROF["l2"] = r2

    out = np.zeros((B, T, C, H, W), np.float32)
    for i in range(8):
        sl = slice(SL * i, SL * (i + 1))
        zz = r2.results[i]["zz"].astype(np.float32)  # (8, 128, 2048)
        for tp in range(8):
            b, t0 = tp // 4, 2 * (tp % 4)
            out[b, t0].reshape(C, S)[:, sl] = zz[tp, 0:64]
            out[b, t0 + 1].reshape(C, S)[:, sl] = zz[tp, 64:128]
    return out
